# revision 18
# baseline (speedup 1.0000x reference)
"""Multi-head attention block (QKV linear -> softmax attention -> proj linear)
for Trainium2, SPMD over 8 NeuronCores.

Sharding: 8 shards = batch (4) x head-group (2 groups of 6 heads).
Each core computes, for its (b, g):
    qkv   = x[b] @ Wqkv[:, cols(g)]            (bf16 matmul, fp32 accum)
    S^T_h = K_h Q_h^T   per head               (keys on partitions)
    P^T_h = exp(SCALE * S^T_h)                 (ACT engine, bf16 out)
    out_h = (P_h @ [V_h | 1]) -> normalize rows by the ones-column sum
    y_g   = concat_h(out_h) @ Wproj[rows(g), :]    (partial, fp32 out)
Host sums the two head-group partials per batch and adds bproj.

Shapes hardcoded: x [4, 2048, 768], Wqkv [768, 2304], Wproj [768, 768].
"""

import os
from contextlib import ExitStack

import numpy as np
import ml_dtypes

import concourse.bass as bass
import concourse.mybir as mybir
import concourse.tile as tile
from concourse import bacc
from concourse.bass_utils import run_bass_kernel_spmd
from concourse.masks import make_identity

B, N, C = 4, 2048, 768
H, D = 12, 64          # total heads, head dim
G = 2                  # head groups (tensor-parallel axis)
HL = H // G            # heads per core = 6
SCALE = D ** -0.5
P = 128
CB = C // P            # 6 contraction blocks
NT = N // P            # 16 row tiles
EG = HL * D            # 384 = per-group width of Q / K / V
NCORES = 8

f32 = mybir.dt.float32
bf16 = mybir.dt.bfloat16

# knobs
PT_BUFS = int(os.environ.get("KRN_PT_BUFS", "25"))
PV_SPLIT = int(os.environ.get("KRN_PV_SPLIT", "1"))  # split-K pieces for PV


def _build_program():
    nc = bacc.Bacc("TRN2", target_bir_lowering=False, debug=False)

    xT = nc.dram_tensor("xT", [C, N], bf16, kind="ExternalInput")           # x[b].T
    wqkv = nc.dram_tensor("wqkv", [C, 3 * EG], bf16, kind="ExternalInput")  # [Qg|Kg|Vg]
    wproj = nc.dram_tensor("wproj", [EG, C], bf16, kind="ExternalInput")    # group rows
    y = nc.dram_tensor("y", [N, C], f32, kind="ExternalOutput")             # partial out

    with tile.TileContext(nc) as tc, ExitStack() as ctx:
        persist = ctx.enter_context(tc.tile_pool(name="persist", bufs=1))
        ptpool = ctx.enter_context(tc.tile_pool(name="ptpool", bufs=PT_BUFS))
        rpool = ctx.enter_context(tc.tile_pool(name="rpool", bufs=8))
        ypool = ctx.enter_context(tc.tile_pool(name="ypool", bufs=2))
        ps_score = ctx.enter_context(tc.tile_pool(name="ps_score", bufs=3, space="PSUM"))
        ps_small = ctx.enter_context(tc.tile_pool(name="ps_small", bufs=2, space="PSUM"))

        identity = persist.tile([P, P], bf16, tag="identity")
        make_identity(nc, identity)

        # ---- loads ----
        xts = []
        for cb in range(CB):
            xt_cb = ptpool.tile([P, N], bf16, tag="pt", name=f"xt{cb}")
            nc.sync.dma_start(xt_cb[:], xT[cb * P : (cb + 1) * P, :])
            xts.append(xt_cb)
        wq_sb = persist.tile([P, CB, 3 * EG], bf16, tag="wq")
        nc.sync.dma_start(wq_sb[:], wqkv[:].rearrange("(cb p) e -> p cb e", p=P))
        wp_sb = persist.tile([P, EG // P, C], bf16, tag="wp")
        nc.sync.dma_start(wp_sb[:], wproj[:].rearrange("(cb p) c -> p cb c", p=P))

        qkT_sb = persist.tile([P, 2 * EG // P, N], bf16, tag="qkT")
        vp_sb = persist.tile([P, NT, HL * (D + 1)], bf16, tag="vp")
        vp4 = vp_sb.rearrange("p m (h c) -> p m h c", c=D + 1)
        nc.vector.memset(vp4[:, :, :, D : D + 1], 1.0)
        og_sb = persist.tile([P, NT, EG], bf16, tag="og")   # heads out [n, ch]

        def emit_qk(eb):
            for nch in range(N // 512):
                qpsum = ps_small.tile([P, 512], f32, tag="sm")
                for cb in range(CB):
                    nc.tensor.matmul(
                        qpsum,
                        wq_sb[:, cb, eb * P : (eb + 1) * P],
                        xts[cb][:, nch * 512 : (nch + 1) * 512],
                        start=(cb == 0),
                        stop=(cb == CB - 1),
                    )
                nc.vector.tensor_copy(
                    qkT_sb[:, eb, nch * 512 : (nch + 1) * 512], qpsum
                )

        def emit_v():
            for mt in range(NT):
                vpsum = ps_small.tile([P, 512], f32, tag="sm")
                for cb in range(CB):
                    nc.tensor.matmul(
                        vpsum[:, :EG],
                        xts[cb][:, mt * P : (mt + 1) * P],
                        wq_sb[:, cb, 2 * EG : 3 * EG],
                        start=(cb == 0),
                        stop=(cb == CB - 1),
                    )
                nc.vector.tensor_copy(
                    vp4[:, mt, :, :D],
                    vpsum[:, :EG].rearrange("p (h d) -> p h d", d=D),
                )

        def emit_scores(h, chaser=None):
            prow = (h % 2) * D
            qblk = h // 2
            kblk = 3 + h // 2
            pts = []
            for mt in range(NT):
                pt = ptpool.tile([P, N], bf16, tag="pt")
                pts.append(pt)
                lhsT = qkT_sb[prow : prow + D, kblk, mt * P : (mt + 1) * P]
                for nch in range(2):
                    spsum = ps_score.tile([P, 1024], f32)
                    for sub in range(2):
                        off = nch * 1024 + sub * 512
                        nc.tensor.matmul(
                            spsum[:, sub * 512 : (sub + 1) * 512],
                            lhsT,
                            qkT_sb[prow : prow + D, qblk, off : off + 512],
                            start=True,
                            stop=True,
                        )
                    nc.scalar.activation(
                        pt[:, nch * 1024 : (nch + 1) * 1024],
                        spsum,
                        mybir.ActivationFunctionType.Exp,
                        scale=SCALE,
                    )
                if chaser is not None:
                    chaser(mt)
            return pts

        def emit_pv_group(h, pts, nt):
            pvpsum = ps_small.tile([P, 512], f32, tag="sm", name="pvpsum")
            for mt in range(NT):
                nc.tensor.matmul(
                    pvpsum[:, : D + 1],
                    pts[mt][:, nt * P : (nt + 1) * P],
                    vp_sb[:, mt, h * (D + 1) : (h + 1) * (D + 1)],
                    start=(mt == 0),
                    stop=(mt == NT - 1),
                )
            r = rpool.tile([P, 1], f32, tag="r", name="r")
            nc.vector.reciprocal(r, pvpsum[:, D : D + 1])
            nc.vector.tensor_scalar(
                og_sb[:, nt, h * D : (h + 1) * D],
                pvpsum[:, :D],
                r,
                None,
                mybir.AluOpType.mult,
            )

        # ---- emission schedule ----
        emit_qk(3)
        emit_qk(0)
        all_pts = [emit_scores(0)]
        emit_qk(4)
        emit_qk(1)
        emit_qk(5)
        emit_qk(2)
        emit_v()
        for h in range(1, HL):
            hh = h

            def chaser(nt, hh=hh):
                emit_pv_group(hh - 1, all_pts[hh - 1], nt)

            all_pts.append(emit_scores(h, chaser=chaser))
        # head-5 PV in the V'-stationary orientation: few weight loads,
        # long streams -- much faster when there is no exp stream to hide in.
        stpool = ctx.enter_context(tc.tile_pool(name="stpool", bufs=1))
        stage = stpool.tile([P, N], bf16, tag="stg")
        h5 = HL - 1
        for cn in range(4):
            ovpsum = ps_small.tile([P, 512], f32, tag="sm", name="ovpsum")
            for mt in range(NT):
                nc.tensor.matmul(
                    ovpsum[: D + 1, :],
                    vp_sb[:, mt, h5 * (D + 1) : (h5 + 1) * (D + 1)],
                    all_pts[h5][mt][:, cn * 512 : (cn + 1) * 512],
                    start=(mt == 0),
                    stop=(mt == NT - 1),
                )
            nc.vector.tensor_copy(
                stage[: D + 1, cn * 512 : (cn + 1) * 512], ovpsum[: D + 1, :]
            )
            for nt in range(4 * cn, 4 * cn + 4):
                tpsumb = ps_small.tile([P, 512], bf16, tag="sm", name="tpsumb")
                nc.tensor.transpose(
                    tpsumb[:, : D + 1],
                    stage[: D + 1, nt * P : (nt + 1) * P],
                    identity[: D + 1, : D + 1],
                )
                r = rpool.tile([P, 1], f32, tag="r", name="rb")
                nc.vector.reciprocal(r, tpsumb[:, D : D + 1])
                nc.vector.tensor_scalar(
                    og_sb[:, nt, h5 * D : (h5 + 1) * D],
                    tpsumb[:, :D],
                    r,
                    None,
                    mybir.AluOpType.mult,
                )

        # ---- transpose heads-out to [ch, n] for proj ----
        ogT_sb = persist.tile([P, EG // P, N], bf16, tag="ogT")
        for nt in range(NT):
            for cb in range(EG // P):
                tpsum = ps_small.tile([P, 512], bf16, tag="sm", name="tpsum")
                nc.tensor.transpose(
                    tpsum[:, :P], og_sb[:, nt, cb * P : (cb + 1) * P], identity
                )
                nc.vector.tensor_copy(
                    ogT_sb[:, cb, nt * P : (nt + 1) * P], tpsum[:, :P]
                )

        # ---- proj ----
        yv = y[:].rearrange("(nt p) c -> p nt c", p=P)
        for nt in range(NT):
            y_sb = ypool.tile([P, C], f32, tag="y", name="y_sb")
            for half in range(2):
                ppsum = ps_small.tile([P, 512], f32, tag="sm", name="ppsum")
                for cb in range(EG // P):
                    nc.tensor.matmul(
                        ppsum[:, :EG],
                        ogT_sb[:, cb, nt * P : (nt + 1) * P],
                        wp_sb[:, cb, half * EG : (half + 1) * EG],
                        start=(cb == 0),
                        stop=(cb == EG // P - 1),
                    )
                nc.vector.tensor_copy(
                    y_sb[:, half * EG : (half + 1) * EG], ppsum[:, :EG]
                )
            nc.sync.dma_start(yv[:, nt], y_sb)

    nc.compile()
    return nc


_PROGRAM = None


def _get_program():
    global _PROGRAM
    if _PROGRAM is None:
        _PROGRAM = _build_program()
    return _PROGRAM


def _shard_inputs(x, Wqkv, Wproj):
    bf = ml_dtypes.bfloat16
    in_maps = []
    for core in range(NCORES):
        b, g = core // G, core % G
        xT = np.ascontiguousarray(x[b].T).astype(bf)
        wg = np.concatenate(
            [
                Wqkv[:, g * EG : (g + 1) * EG],
                Wqkv[:, C + g * EG : C + (g + 1) * EG],
                Wqkv[:, 2 * C + g * EG : 2 * C + (g + 1) * EG],
            ],
            axis=1,
        ).astype(bf)
        wp = np.ascontiguousarray(Wproj[g * EG : (g + 1) * EG, :]).astype(bf)
        in_maps.append({"xT": xT, "wqkv": wg, "wproj": wp})
    return in_maps


def _run(x, Wqkv, Wproj, bproj, trace=False):
    nc = _get_program()
    in_maps = _shard_inputs(x, Wqkv, Wproj)
    res = run_bass_kernel_spmd(nc, in_maps, list(range(NCORES)), trace=trace)
    out = np.empty((B, N, C), np.float32)
    for b in range(B):
        out[b] = res.results[b * G]["y"] + res.results[b * G + 1]["y"] + bproj
    return out, res


def kernel(x, Wqkv, Wproj, bproj):
    x = np.asarray(x, np.float32)
    Wqkv = np.asarray(Wqkv, np.float32)
    Wproj = np.asarray(Wproj, np.float32)
    bproj = np.asarray(bproj, np.float32)
    out, _ = _run(x, Wqkv, Wproj, bproj)
    return out


# revision 19
# speedup vs baseline: 1.1493x; 1.1493x over previous
"""Multi-head attention block (QKV linear -> softmax attention -> proj linear)
for Trainium2, SPMD over 8 NeuronCores.

Sharding: 8 shards = batch (4) x head-group (2 groups of 6 heads).
Each core computes, for its (b, g):
    qkv   = x[b] @ Wqkv[:, cols(g)]            (bf16 matmul, fp32 accum)
    S^T_h = K_h Q_h^T   per head               (keys on partitions)
    P^T_h = exp(SCALE * S^T_h)                 (ACT engine, bf16 out)
    out_h = (P_h @ [V_h | 1]) -> normalize rows by the ones-column sum
    y_g   = concat_h(out_h) @ Wproj[rows(g), :]    (partial, fp32 out)
Host sums the two head-group partials per batch and adds bproj.

Shapes hardcoded: x [4, 2048, 768], Wqkv [768, 2304], Wproj [768, 768].
"""

import os
from contextlib import ExitStack

import numpy as np
import ml_dtypes

import concourse.bass as bass
import concourse.mybir as mybir
import concourse.tile as tile
from concourse import bacc
from concourse.bass_utils import run_bass_kernel_spmd
from concourse.masks import make_identity

B, N, C = 4, 2048, 768
H, D = 12, 64          # total heads, head dim
G = 2                  # head groups (tensor-parallel axis)
HL = H // G            # heads per core = 6
SCALE = D ** -0.5
P = 128
CB = C // P            # 6 contraction blocks
NT = N // P            # 16 row tiles
EG = HL * D            # 384 = per-group width of Q / K / V
NCORES = 8

f32 = mybir.dt.float32
bf16 = mybir.dt.bfloat16

# knobs
PT_BUFS = int(os.environ.get("KRN_PT_BUFS", "28"))
PV_SPLIT = int(os.environ.get("KRN_PV_SPLIT", "1"))  # split-K pieces for PV


def _build_program():
    nc = bacc.Bacc("TRN2", target_bir_lowering=False, debug=False)

    xT = nc.dram_tensor("xT", [C, N], bf16, kind="ExternalInput")           # x[b].T
    wqkv = nc.dram_tensor("wqkv", [C, 3 * EG], bf16, kind="ExternalInput")  # [Qg|Kg|Vg]
    wproj = nc.dram_tensor("wproj", [EG, C], bf16, kind="ExternalInput")    # group rows
    y = nc.dram_tensor("y", [N, C], f32, kind="ExternalOutput")             # partial out

    with tile.TileContext(nc) as tc, ExitStack() as ctx:
        persist = ctx.enter_context(tc.tile_pool(name="persist", bufs=1))
        ptpool = ctx.enter_context(tc.tile_pool(name="ptpool", bufs=PT_BUFS))
        rpool = ctx.enter_context(tc.tile_pool(name="rpool", bufs=8))
        ypool = ctx.enter_context(tc.tile_pool(name="ypool", bufs=3))
        ps_score = ctx.enter_context(tc.tile_pool(name="ps_score", bufs=3, space="PSUM"))
        ps_small = ctx.enter_context(tc.tile_pool(name="ps_small", bufs=2, space="PSUM"))

        identity = persist.tile([P, P], bf16, tag="identity")
        make_identity(nc, identity)

        # ---- loads ----
        xts = []
        for cb in range(CB):
            xt_cb = ptpool.tile([P, N], bf16, tag="pt", name=f"xt{cb}")
            nc.sync.dma_start(xt_cb[:], xT[cb * P : (cb + 1) * P, :])
            xts.append(xt_cb)
        wq_sb = persist.tile([P, CB, 3 * EG], bf16, tag="wq")
        nc.sync.dma_start(wq_sb[:], wqkv[:].rearrange("(cb p) e -> p cb e", p=P))
        wp_sb = persist.tile([P, EG // P, C], bf16, tag="wp")
        nc.sync.dma_start(wp_sb[:], wproj[:].rearrange("(cb p) c -> p cb c", p=P))

        qkT_sb = persist.tile([P, 2 * EG // P, N], bf16, tag="qkT")
        vp_sb = persist.tile([P, NT, HL * (D + 1)], bf16, tag="vp")
        vp4 = vp_sb.rearrange("p m (h c) -> p m h c", c=D + 1)
        nc.vector.memset(vp4[:, :, :, D : D + 1], 1.0)
        og_sb = persist.tile([P, NT, EG], bf16, tag="og")   # heads out [n, ch]

        def emit_qk(eb):
            for nch in range(N // 512):
                qpsum = ps_small.tile([P, 512], f32, tag="sm")
                for cb in range(CB):
                    nc.tensor.matmul(
                        qpsum,
                        wq_sb[:, cb, eb * P : (eb + 1) * P],
                        xts[cb][:, nch * 512 : (nch + 1) * 512],
                        start=(cb == 0),
                        stop=(cb == CB - 1),
                    )
                nc.vector.tensor_copy(
                    qkT_sb[:, eb, nch * 512 : (nch + 1) * 512], qpsum
                )

        def emit_v():
            for mt in range(NT):
                vpsum = ps_small.tile([P, 512], f32, tag="sm")
                for cb in range(CB):
                    nc.tensor.matmul(
                        vpsum[:, :EG],
                        xts[cb][:, mt * P : (mt + 1) * P],
                        wq_sb[:, cb, 2 * EG : 3 * EG],
                        start=(cb == 0),
                        stop=(cb == CB - 1),
                    )
                nc.vector.tensor_copy(
                    vp4[:, mt, :, :D],
                    vpsum[:, :EG].rearrange("p (h d) -> p h d", d=D),
                )

        def emit_scores(h, chaser=None):
            prow = (h % 2) * D
            qblk = h // 2
            kblk = 3 + h // 2
            pts = []
            for mt in range(NT):
                pt = ptpool.tile([P, N], bf16, tag="pt")
                pts.append(pt)
                lhsT = qkT_sb[prow : prow + D, kblk, mt * P : (mt + 1) * P]
                for nch in range(2):
                    spsum = ps_score.tile([P, 1024], f32, tag="spsum")
                    for sub in range(2):
                        off = nch * 1024 + sub * 512
                        nc.tensor.matmul(
                            spsum[:, sub * 512 : (sub + 1) * 512],
                            lhsT,
                            qkT_sb[prow : prow + D, qblk, off : off + 512],
                            start=True,
                            stop=True,
                        )
                    nc.scalar.activation(
                        pt[:, nch * 1024 : (nch + 1) * 1024],
                        spsum,
                        mybir.ActivationFunctionType.Exp,
                        scale=SCALE,
                    )
                if chaser is not None:
                    chaser(mt)
            return pts

        def emit_pv_group(h, pts, nt):
            pvpsum = ps_small.tile([P, 512], f32, tag="sm", name="pvpsum")
            for mt in range(NT):
                nc.tensor.matmul(
                    pvpsum[:, : D + 1],
                    pts[mt][:, nt * P : (nt + 1) * P],
                    vp_sb[:, mt, h * (D + 1) : (h + 1) * (D + 1)],
                    start=(mt == 0),
                    stop=(mt == NT - 1),
                )
            r = rpool.tile([P, 1], f32, tag="r", name="r")
            nc.vector.reciprocal(r, pvpsum[:, D : D + 1])
            nc.vector.tensor_scalar(
                og_sb[:, nt, h * D : (h + 1) * D],
                pvpsum[:, :D],
                r,
                None,
                mybir.AluOpType.mult,
            )

        # ---- emission schedule ----
        emit_qk(3)
        emit_qk(0)
        all_pts = [emit_scores(0)]
        emit_qk(4)
        emit_qk(1)
        emit_qk(5)
        emit_qk(2)
        emit_v()
        for h in range(1, HL):
            hh = h

            def chaser(nt, hh=hh):
                emit_pv_group(hh - 1, all_pts[hh - 1], nt)

            all_pts.append(emit_scores(h, chaser=chaser))
        for nt in range(NT):
            emit_pv_group(HL - 1, all_pts[HL - 1], nt)

        # ---- transpose heads-out to [ch, n] for proj ----
        # ogT lives in three "pt"-tagged tiles: by the tail only head-5's
        # P^T tiles are live, so the slots are free.
        ogTs = [ptpool.tile([P, N], bf16, tag="pt", name=f"ogT{cb}")
                for cb in range(EG // P)]
        for nt in range(NT):
            for cb in range(EG // P):
                tpsum = ps_small.tile([P, 512], bf16, tag="sm", name="tpsum")
                nc.tensor.transpose(
                    tpsum[:, :P], og_sb[:, nt, cb * P : (cb + 1) * P], identity
                )
                nc.vector.tensor_copy(
                    ogTs[cb][:, nt * P : (nt + 1) * P], tpsum[:, :P]
                )

        # ---- proj ----
        yv = y[:].rearrange("(nt p) c -> p nt c", p=P)
        for nt in range(NT):
            y_sb = ypool.tile([P, C], f32, tag="y", name="y_sb")
            for half in range(2):
                ppsum = ps_score.tile([P, 1024], f32, tag="spsum", name="ppsum")
                for cb in range(EG // P):
                    nc.tensor.matmul(
                        ppsum[:, :EG],
                        ogTs[cb][:, nt * P : (nt + 1) * P],
                        wp_sb[:, cb, half * EG : (half + 1) * EG],
                        start=(cb == 0),
                        stop=(cb == EG // P - 1),
                    )
                nc.vector.tensor_copy(
                    y_sb[:, half * EG : (half + 1) * EG], ppsum[:, :EG]
                )
            nc.sync.dma_start(yv[:, nt], y_sb)

    nc.compile()
    return nc


_PROGRAM = None


def _get_program():
    global _PROGRAM
    if _PROGRAM is None:
        _PROGRAM = _build_program()
    return _PROGRAM


def _shard_inputs(x, Wqkv, Wproj):
    bf = ml_dtypes.bfloat16
    in_maps = []
    for core in range(NCORES):
        b, g = core // G, core % G
        xT = np.ascontiguousarray(x[b].T).astype(bf)
        wg = np.concatenate(
            [
                Wqkv[:, g * EG : (g + 1) * EG],
                Wqkv[:, C + g * EG : C + (g + 1) * EG],
                Wqkv[:, 2 * C + g * EG : 2 * C + (g + 1) * EG],
            ],
            axis=1,
        ).astype(bf)
        wp = np.ascontiguousarray(Wproj[g * EG : (g + 1) * EG, :]).astype(bf)
        in_maps.append({"xT": xT, "wqkv": wg, "wproj": wp})
    return in_maps


def _run(x, Wqkv, Wproj, bproj, trace=False):
    nc = _get_program()
    in_maps = _shard_inputs(x, Wqkv, Wproj)
    res = run_bass_kernel_spmd(nc, in_maps, list(range(NCORES)), trace=trace)
    out = np.empty((B, N, C), np.float32)
    for b in range(B):
        out[b] = res.results[b * G]["y"] + res.results[b * G + 1]["y"] + bproj
    return out, res


def kernel(x, Wqkv, Wproj, bproj):
    x = np.asarray(x, np.float32)
    Wqkv = np.asarray(Wqkv, np.float32)
    Wproj = np.asarray(Wproj, np.float32)
    bproj = np.asarray(bproj, np.float32)
    out, _ = _run(x, Wqkv, Wproj, bproj)
    return out


# revision 20
# speedup vs baseline: 1.1653x; 1.0139x over previous
"""Multi-head attention block (QKV linear -> softmax attention -> proj linear)
for Trainium2, SPMD over 8 NeuronCores.

Sharding: 8 shards = batch (4) x head-group (2 groups of 6 heads).
Each core computes, for its (b, g):
    qkv   = x[b] @ Wqkv[:, cols(g)]            (bf16 matmul, fp32 accum)
    S^T_h = K_h Q_h^T   per head               (keys on partitions)
    P^T_h = exp(SCALE * S^T_h)                 (ACT engine, bf16 out)
    out_h = (P_h @ [V_h | 1]) -> normalize rows by the ones-column sum
    y_g   = concat_h(out_h) @ Wproj[rows(g), :]    (partial, fp32 out)
Host sums the two head-group partials per batch and adds bproj.

Shapes hardcoded: x [4, 2048, 768], Wqkv [768, 2304], Wproj [768, 768].
"""

import os
from contextlib import ExitStack

import numpy as np
import ml_dtypes

import concourse.bass as bass
import concourse.mybir as mybir
import concourse.tile as tile
from concourse import bacc
from concourse.bass_utils import run_bass_kernel_spmd
from concourse.masks import make_identity

B, N, C = 4, 2048, 768
H, D = 12, 64          # total heads, head dim
G = 2                  # head groups (tensor-parallel axis)
HL = H // G            # heads per core = 6
SCALE = D ** -0.5
P = 128
CB = C // P            # 6 contraction blocks
NT = N // P            # 16 row tiles
EG = HL * D            # 384 = per-group width of Q / K / V
NCORES = 8

f32 = mybir.dt.float32
bf16 = mybir.dt.bfloat16

# knobs
PT_BUFS = int(os.environ.get("KRN_PT_BUFS", "28"))
PV_SPLIT = int(os.environ.get("KRN_PV_SPLIT", "1"))  # split-K pieces for PV


def _build_program():
    nc = bacc.Bacc("TRN2", target_bir_lowering=False, debug=False)

    xT = nc.dram_tensor("xT", [C, N], bf16, kind="ExternalInput")           # x[b].T
    wqkv = nc.dram_tensor("wqkv", [C, 3 * EG], bf16, kind="ExternalInput")  # [Qg|Kg|Vg]
    wproj = nc.dram_tensor("wproj", [EG, C], bf16, kind="ExternalInput")    # group rows
    y = nc.dram_tensor("y", [N, C], f32, kind="ExternalOutput")             # partial out

    with tile.TileContext(nc) as tc, ExitStack() as ctx:
        persist = ctx.enter_context(tc.tile_pool(name="persist", bufs=1))
        ptpool = ctx.enter_context(tc.tile_pool(name="ptpool", bufs=PT_BUFS))
        rpool = ctx.enter_context(tc.tile_pool(name="rpool", bufs=8))
        ypool = ctx.enter_context(tc.tile_pool(name="ypool", bufs=3))
        ps_score = ctx.enter_context(tc.tile_pool(name="ps_score", bufs=3, space="PSUM"))
        ps_small = ctx.enter_context(tc.tile_pool(name="ps_small", bufs=2, space="PSUM"))

        identity = persist.tile([P, P], bf16, tag="identity")
        make_identity(nc, identity)

        # ---- loads ----
        xts = []
        for cb in range(CB):
            xt_cb = ptpool.tile([P, N], bf16, tag="pt", name=f"xt{cb}")
            nc.sync.dma_start(xt_cb[:], xT[cb * P : (cb + 1) * P, :])
            xts.append(xt_cb)
        wq_sb = persist.tile([P, CB, 3 * EG], bf16, tag="wq")
        nc.sync.dma_start(wq_sb[:], wqkv[:].rearrange("(cb p) e -> p cb e", p=P))
        wp_sb = persist.tile([P, EG // P, C], bf16, tag="wp")
        nc.sync.dma_start(wp_sb[:], wproj[:].rearrange("(cb p) c -> p cb c", p=P))

        qkT_sb = persist.tile([P, 2 * EG // P, N], bf16, tag="qkT")
        vp_sb = persist.tile([P, NT, HL * (D + 1)], bf16, tag="vp")
        vp4 = vp_sb.rearrange("p m (h c) -> p m h c", c=D + 1)
        nc.vector.memset(vp4[:, :, :, D : D + 1], 1.0)
        og_sb = persist.tile([P, NT, EG], bf16, tag="og")   # heads out [n, ch]

        def qk_chunk(eb, nch):
            def go():
                qpsum = ps_small.tile([P, 512], f32, tag="sm", name="qpsum")
                for cb in range(CB):
                    nc.tensor.matmul(
                        qpsum,
                        wq_sb[:, cb, eb * P : (eb + 1) * P],
                        xts[cb][:, nch * 512 : (nch + 1) * 512],
                        start=(cb == 0),
                        stop=(cb == CB - 1),
                    )
                nc.vector.tensor_copy(
                    qkT_sb[:, eb, nch * 512 : (nch + 1) * 512], qpsum
                )
            return go

        def v_group(mt):
            def go():
                vpsum = ps_small.tile([P, 512], f32, tag="sm", name="vpsum")
                for cb in range(CB):
                    nc.tensor.matmul(
                        vpsum[:, :EG],
                        xts[cb][:, mt * P : (mt + 1) * P],
                        wq_sb[:, cb, 2 * EG : 3 * EG],
                        start=(cb == 0),
                        stop=(cb == CB - 1),
                    )
                nc.vector.tensor_copy(
                    vp4[:, mt, :, :D],
                    vpsum[:, :EG].rearrange("p (h d) -> p h d", d=D),
                )
            return go

        def emit_scores(h, work=None, pts=None, nchs=(0, 1)):
            """Scores + exp for one head; `work` closures are spread evenly
            through the emission so no block stalls the PE queue. `nchs`
            selects which halves of the key rows to emit (two-pass head 0)."""
            prow = (h % 2) * D
            qblk = h // 2
            kblk = 3 + h // 2
            if pts is None:
                pts = [ptpool.tile([P, N], bf16, tag="pt", name=f"pt{h}_{mt}")
                       for mt in range(NT)]
            work = work or []
            wi = 0
            for mt in range(NT):
                lhsT = qkT_sb[prow : prow + D, kblk, mt * P : (mt + 1) * P]
                for nch in nchs:
                    spsum = ps_score.tile([P, 1024], f32, tag="spsum")
                    for sub in range(2):
                        off = nch * 1024 + sub * 512
                        nc.tensor.matmul(
                            spsum[:, sub * 512 : (sub + 1) * 512],
                            lhsT,
                            qkT_sb[prow : prow + D, qblk, off : off + 512],
                            start=True,
                            stop=True,
                        )
                    nc.scalar.activation(
                        pt[:, nch * 1024 : (nch + 1) * 1024] if False else pts[mt][:, nch * 1024 : (nch + 1) * 1024],
                        spsum,
                        mybir.ActivationFunctionType.Exp,
                        scale=SCALE,
                    )
                hi = (mt + 1) * len(work) // NT
                while wi < hi:
                    work[wi]()
                    wi += 1
            return pts

        def emit_pv_group(h, pts, nt):
            pvpsum = ps_small.tile([P, 512], f32, tag="sm", name="pvpsum")
            for mt in range(NT):
                nc.tensor.matmul(
                    pvpsum[:, : D + 1],
                    pts[mt][:, nt * P : (nt + 1) * P],
                    vp_sb[:, mt, h * (D + 1) : (h + 1) * (D + 1)],
                    start=(mt == 0),
                    stop=(mt == NT - 1),
                )
            r = rpool.tile([P, 1], f32, tag="r", name="r")
            nc.vector.reciprocal(r, pvpsum[:, D : D + 1])
            nc.vector.tensor_scalar(
                og_sb[:, nt, h * D : (h + 1) * D],
                pvpsum[:, :D],
                r,
                None,
                mybir.AluOpType.mult,
            )

        # ---- emission schedule ----
        # Heads 2h and 2h+1 share Q/K blocks (eb h and 3+h), so only K3+Q0
        # are needed before heads 0 AND 1. Head-0 scores start after just
        # three QK chunks; the rest of QKV rides inside the exp stream.
        qk_chunk(3, 0)()
        qk_chunk(0, 0)()
        qk_chunk(0, 1)()
        pts0 = [ptpool.tile([P, N], bf16, tag="pt", name=f"pt0_{mt}")
                for mt in range(NT)]
        workA = [qk_chunk(3, 1), qk_chunk(0, 2), qk_chunk(3, 2),
                 qk_chunk(0, 3), qk_chunk(3, 3)] + [v_group(m) for m in range(8)]
        emit_scores(0, work=workA, pts=pts0, nchs=(0,))
        workB = [v_group(m) for m in range(8, NT)]
        emit_scores(0, work=workB, pts=pts0, nchs=(1,))
        all_pts = [pts0]

        def pv_work(h, pts):
            return [(lambda nt=nt: emit_pv_group(h, pts, nt)) for nt in range(NT)]

        # ogT lives in three "pt"-tagged tiles; transposes for a column pair
        # chase two heads after the pair completes.
        ogTs = [ptpool.tile([P, N], bf16, tag="pt", name=f"ogT{cb}")
                for cb in range(EG // P)]

        def ogT_work(cb):
            def one(nt):
                def go():
                    tpsum = ps_small.tile([P, 512], bf16, tag="sm", name="tpsum")
                    nc.tensor.transpose(
                        tpsum[:, :P], og_sb[:, nt, cb * P : (cb + 1) * P], identity
                    )
                    nc.vector.tensor_copy(
                        ogTs[cb][:, nt * P : (nt + 1) * P], tpsum[:, :P]
                    )
                return go
            return [one(nt) for nt in range(NT)]

        plans = {
            1: [qk_chunk(4, i) for i in range(4)] + [qk_chunk(1, i) for i in range(4)],
            3: [qk_chunk(5, i) for i in range(4)] + [qk_chunk(2, i) for i in range(4)],
            4: ogT_work(0),
            5: ogT_work(1),
        }
        for h in range(1, HL):
            work = pv_work(h - 1, all_pts[h - 1]) + plans.get(h, [])
            all_pts.append(emit_scores(h, work=work))
        for nt in range(NT):
            emit_pv_group(HL - 1, all_pts[HL - 1], nt)
        for go in ogT_work(2):
            go()

        # ---- proj ----
        yv = y[:].rearrange("(nt p) c -> p nt c", p=P)
        for nt in range(NT):
            y_sb = ypool.tile([P, C], f32, tag="y", name="y_sb")
            for half in range(2):
                ppsum = ps_score.tile([P, 1024], f32, tag="spsum", name="ppsum")
                for cb in range(EG // P):
                    nc.tensor.matmul(
                        ppsum[:, :EG],
                        ogTs[cb][:, nt * P : (nt + 1) * P],
                        wp_sb[:, cb, half * EG : (half + 1) * EG],
                        start=(cb == 0),
                        stop=(cb == EG // P - 1),
                    )
                nc.vector.tensor_copy(
                    y_sb[:, half * EG : (half + 1) * EG], ppsum[:, :EG]
                )
            nc.sync.dma_start(yv[:, nt], y_sb)

    nc.compile()
    return nc


_PROGRAM = None


def _get_program():
    global _PROGRAM
    if _PROGRAM is None:
        _PROGRAM = _build_program()
    return _PROGRAM


def _shard_inputs(x, Wqkv, Wproj):
    bf = ml_dtypes.bfloat16
    in_maps = []
    for core in range(NCORES):
        b, g = core // G, core % G
        xT = np.ascontiguousarray(x[b].T).astype(bf)
        wg = np.concatenate(
            [
                Wqkv[:, g * EG : (g + 1) * EG],
                Wqkv[:, C + g * EG : C + (g + 1) * EG],
                Wqkv[:, 2 * C + g * EG : 2 * C + (g + 1) * EG],
            ],
            axis=1,
        ).astype(bf)
        wp = np.ascontiguousarray(Wproj[g * EG : (g + 1) * EG, :]).astype(bf)
        in_maps.append({"xT": xT, "wqkv": wg, "wproj": wp})
    return in_maps


def _run(x, Wqkv, Wproj, bproj, trace=False):
    nc = _get_program()
    in_maps = _shard_inputs(x, Wqkv, Wproj)
    res = run_bass_kernel_spmd(nc, in_maps, list(range(NCORES)), trace=trace)
    out = np.empty((B, N, C), np.float32)
    for b in range(B):
        out[b] = res.results[b * G]["y"] + res.results[b * G + 1]["y"] + bproj
    return out, res


def kernel(x, Wqkv, Wproj, bproj):
    x = np.asarray(x, np.float32)
    Wqkv = np.asarray(Wqkv, np.float32)
    Wproj = np.asarray(Wproj, np.float32)
    bproj = np.asarray(bproj, np.float32)
    out, _ = _run(x, Wqkv, Wproj, bproj)
    return out


# revision 21
# speedup vs baseline: 1.1774x; 1.0104x over previous
"""Multi-head attention block (QKV linear -> softmax attention -> proj linear)
for Trainium2, SPMD over 8 NeuronCores.

Sharding: 8 shards = batch (4) x head-group (2 groups of 6 heads).
Each core computes, for its (b, g):
    qkv   = x[b] @ Wqkv[:, cols(g)]            (bf16 matmul, fp32 accum)
    S^T_h = K_h Q_h^T   per head               (keys on partitions)
    P^T_h = exp(SCALE * S^T_h)                 (ACT engine, bf16 out)
    out_h = (P_h @ [V_h | 1]) -> normalize rows by the ones-column sum
    y_g   = concat_h(out_h) @ Wproj[rows(g), :]    (partial, fp32 out)
Host sums the two head-group partials per batch and adds bproj.

Shapes hardcoded: x [4, 2048, 768], Wqkv [768, 2304], Wproj [768, 768].
"""

import os
from contextlib import ExitStack

import numpy as np
import ml_dtypes

import concourse.bass as bass
import concourse.mybir as mybir
import concourse.tile as tile
from concourse import bacc
from concourse.bass_utils import run_bass_kernel_spmd
from concourse.masks import make_identity

B, N, C = 4, 2048, 768
H, D = 12, 64          # total heads, head dim
G = 2                  # head groups (tensor-parallel axis)
HL = H // G            # heads per core = 6
SCALE = D ** -0.5
P = 128
CB = C // P            # 6 contraction blocks
NT = N // P            # 16 row tiles
EG = HL * D            # 384 = per-group width of Q / K / V
NCORES = 8

f32 = mybir.dt.float32
bf16 = mybir.dt.bfloat16

# knobs
PT_BUFS = int(os.environ.get("KRN_PT_BUFS", "28"))
PV_SPLIT = int(os.environ.get("KRN_PV_SPLIT", "1"))  # split-K pieces for PV


def _build_program():
    nc = bacc.Bacc("TRN2", target_bir_lowering=False, debug=False)

    xT = nc.dram_tensor("xT", [C, N], bf16, kind="ExternalInput")           # x[b].T
    wqkv = nc.dram_tensor("wqkv", [C, 3 * EG], bf16, kind="ExternalInput")  # [Qg|Kg|Vg]
    wproj = nc.dram_tensor("wproj", [EG, C], bf16, kind="ExternalInput")    # group rows
    y = nc.dram_tensor("y", [N, C], f32, kind="ExternalOutput")             # partial out

    with tile.TileContext(nc) as tc, ExitStack() as ctx:
        persist = ctx.enter_context(tc.tile_pool(name="persist", bufs=1))
        ptpool = ctx.enter_context(tc.tile_pool(name="ptpool", bufs=PT_BUFS))
        rpool = ctx.enter_context(tc.tile_pool(name="rpool", bufs=8))
        ypool = ctx.enter_context(tc.tile_pool(name="ypool", bufs=3))
        ps_score = ctx.enter_context(tc.tile_pool(name="ps_score", bufs=3, space="PSUM"))
        ps_small = ctx.enter_context(tc.tile_pool(name="ps_small", bufs=2, space="PSUM"))

        identity = persist.tile([P, P], bf16, tag="identity")
        make_identity(nc, identity)

        # ---- loads ----
        wq_sb = persist.tile([P, CB, 3 * EG], bf16, tag="wq")
        nc.sync.dma_start(wq_sb[:], wqkv[:].rearrange("(cb p) e -> p cb e", p=P))
        xts = []
        for cb in range(CB):
            xt_cb = ptpool.tile([P, N], bf16, tag="pt", name=f"xt{cb}")
            nc.sync.dma_start(xt_cb[:], xT[cb * P : (cb + 1) * P, :])
            xts.append(xt_cb)
        wp_sb = persist.tile([P, EG // P, C], bf16, tag="wp")
        nc.sync.dma_start(wp_sb[:], wproj[:].rearrange("(cb p) c -> p cb c", p=P))

        qkT_sb = persist.tile([P, 2 * EG // P, N], bf16, tag="qkT")
        vp_sb = persist.tile([P, NT, HL * (D + 1)], bf16, tag="vp")
        vp4 = vp_sb.rearrange("p m (h c) -> p m h c", c=D + 1)
        nc.vector.memset(vp4[:, :, :, D : D + 1], 1.0)
        og_sb = persist.tile([P, NT, EG], bf16, tag="og")   # heads out [n, ch]

        def qk_chunk(eb, nch):
            def go():
                qpsum = ps_small.tile([P, 512], f32, tag="sm", name="qpsum")
                for cb in range(CB):
                    nc.tensor.matmul(
                        qpsum,
                        wq_sb[:, cb, eb * P : (eb + 1) * P],
                        xts[cb][:, nch * 512 : (nch + 1) * 512],
                        start=(cb == 0),
                        stop=(cb == CB - 1),
                    )
                nc.vector.tensor_copy(
                    qkT_sb[:, eb, nch * 512 : (nch + 1) * 512], qpsum
                )
            return go

        def v_group(mt):
            def go():
                vpsum = ps_small.tile([P, 512], f32, tag="sm", name="vpsum")
                for cb in range(CB):
                    nc.tensor.matmul(
                        vpsum[:, :EG],
                        xts[cb][:, mt * P : (mt + 1) * P],
                        wq_sb[:, cb, 2 * EG : 3 * EG],
                        start=(cb == 0),
                        stop=(cb == CB - 1),
                    )
                nc.vector.tensor_copy(
                    vp4[:, mt, :, :D],
                    vpsum[:, :EG].rearrange("p (h d) -> p h d", d=D),
                )
            return go

        def emit_scores(h, work=None, pts=None, nchs=(0, 1)):
            """Scores + exp for one head; `work` closures are spread evenly
            through the emission so no block stalls the PE queue. `nchs`
            selects which halves of the key rows to emit (two-pass head 0)."""
            prow = (h % 2) * D
            qblk = h // 2
            kblk = 3 + h // 2
            if pts is None:
                pts = [ptpool.tile([P, N], bf16, tag="pt", name=f"pt{h}_{mt}")
                       for mt in range(NT)]
            work = work or []
            wi = 0
            for mt in range(NT):
                lhsT = qkT_sb[prow : prow + D, kblk, mt * P : (mt + 1) * P]
                for nch in nchs:
                    spsum = ps_score.tile([P, 1024], f32, tag="spsum")
                    for sub in range(2):
                        off = nch * 1024 + sub * 512
                        nc.tensor.matmul(
                            spsum[:, sub * 512 : (sub + 1) * 512],
                            lhsT,
                            qkT_sb[prow : prow + D, qblk, off : off + 512],
                            start=True,
                            stop=True,
                        )
                    nc.scalar.activation(
                        pt[:, nch * 1024 : (nch + 1) * 1024] if False else pts[mt][:, nch * 1024 : (nch + 1) * 1024],
                        spsum,
                        mybir.ActivationFunctionType.Exp,
                        scale=SCALE,
                    )
                hi = (mt + 1) * len(work) // NT
                while wi < hi:
                    work[wi]()
                    wi += 1
            return pts

        def emit_pv_group(h, pts, nt):
            pvpsum = ps_small.tile([P, 512], f32, tag="sm", name="pvpsum")
            for mt in range(NT):
                nc.tensor.matmul(
                    pvpsum[:, : D + 1],
                    pts[mt][:, nt * P : (nt + 1) * P],
                    vp_sb[:, mt, h * (D + 1) : (h + 1) * (D + 1)],
                    start=(mt == 0),
                    stop=(mt == NT - 1),
                )
            r = rpool.tile([P, 1], f32, tag="r", name="r")
            nc.vector.reciprocal(r, pvpsum[:, D : D + 1])
            nc.vector.tensor_scalar(
                og_sb[:, nt, h * D : (h + 1) * D],
                pvpsum[:, :D],
                r,
                None,
                mybir.AluOpType.mult,
            )

        # ---- emission schedule ----
        # Heads 2h and 2h+1 share Q/K blocks (eb h and 3+h), so only K3+Q0
        # are needed before heads 0 AND 1. Head-0 scores start after just
        # three QK chunks; the rest of QKV rides inside the exp stream.
        qk_chunk(3, 0)()
        qk_chunk(0, 0)()
        qk_chunk(0, 1)()
        pts0 = [ptpool.tile([P, N], bf16, tag="pt", name=f"pt0_{mt}")
                for mt in range(NT)]
        workA = [qk_chunk(3, 1), qk_chunk(0, 2), qk_chunk(3, 2),
                 qk_chunk(0, 3), qk_chunk(3, 3)] + [v_group(m) for m in range(8)]
        emit_scores(0, work=workA, pts=pts0, nchs=(0,))
        workB = [v_group(m) for m in range(8, NT)]
        emit_scores(0, work=workB, pts=pts0, nchs=(1,))
        all_pts = [pts0]

        def pv_work(h, pts):
            return [(lambda nt=nt: emit_pv_group(h, pts, nt)) for nt in range(NT)]

        # ogT lives in three "pt"-tagged tiles; transposes for a column pair
        # chase two heads after the pair completes.
        ogTs = [ptpool.tile([P, N], bf16, tag="pt", name=f"ogT{cb}")
                for cb in range(EG // P)]

        def ogT_work(cb):
            def one(nt):
                def go():
                    tpsum = ps_small.tile([P, 512], bf16, tag="sm", name="tpsum")
                    nc.tensor.transpose(
                        tpsum[:, :P], og_sb[:, nt, cb * P : (cb + 1) * P], identity
                    )
                    nc.vector.tensor_copy(
                        ogTs[cb][:, nt * P : (nt + 1) * P], tpsum[:, :P]
                    )
                return go
            return [one(nt) for nt in range(NT)]

        plans = {
            1: [qk_chunk(4, i) for i in range(4)] + [qk_chunk(1, i) for i in range(4)],
            3: [qk_chunk(5, i) for i in range(4)] + [qk_chunk(2, i) for i in range(4)],
            4: ogT_work(0),
            5: ogT_work(1),
        }
        for h in range(1, HL):
            work = pv_work(h - 1, all_pts[h - 1]) + plans.get(h, [])
            all_pts.append(emit_scores(h, work=work))
        for nt in range(NT):
            emit_pv_group(HL - 1, all_pts[HL - 1], nt)
        for go in ogT_work(2):
            go()

        # ---- proj ----
        yv = y[:].rearrange("(nt p) c -> p nt c", p=P)
        for nt in range(NT):
            y_sb = ypool.tile([P, C], f32, tag="y", name="y_sb")
            for half in range(2):
                ppsum = ps_score.tile([P, 1024], f32, tag="spsum", name="ppsum")
                for cb in range(EG // P):
                    nc.tensor.matmul(
                        ppsum[:, :EG],
                        ogTs[cb][:, nt * P : (nt + 1) * P],
                        wp_sb[:, cb, half * EG : (half + 1) * EG],
                        start=(cb == 0),
                        stop=(cb == EG // P - 1),
                    )
                nc.vector.tensor_copy(
                    y_sb[:, half * EG : (half + 1) * EG], ppsum[:, :EG]
                )
            nc.sync.dma_start(yv[:, nt], y_sb)

    nc.compile()
    return nc


_PROGRAM = None


def _get_program():
    global _PROGRAM
    if _PROGRAM is None:
        _PROGRAM = _build_program()
    return _PROGRAM


def _shard_inputs(x, Wqkv, Wproj):
    bf = ml_dtypes.bfloat16
    in_maps = []
    for core in range(NCORES):
        b, g = core // G, core % G
        xT = np.ascontiguousarray(x[b].T).astype(bf)
        wg = np.concatenate(
            [
                Wqkv[:, g * EG : (g + 1) * EG],
                Wqkv[:, C + g * EG : C + (g + 1) * EG],
                Wqkv[:, 2 * C + g * EG : 2 * C + (g + 1) * EG],
            ],
            axis=1,
        ).astype(bf)
        wp = np.ascontiguousarray(Wproj[g * EG : (g + 1) * EG, :]).astype(bf)
        in_maps.append({"xT": xT, "wqkv": wg, "wproj": wp})
    return in_maps


def _run(x, Wqkv, Wproj, bproj, trace=False):
    nc = _get_program()
    in_maps = _shard_inputs(x, Wqkv, Wproj)
    res = run_bass_kernel_spmd(nc, in_maps, list(range(NCORES)), trace=trace)
    out = np.empty((B, N, C), np.float32)
    for b in range(B):
        out[b] = res.results[b * G]["y"] + res.results[b * G + 1]["y"] + bproj
    return out, res


def kernel(x, Wqkv, Wproj, bproj):
    x = np.asarray(x, np.float32)
    Wqkv = np.asarray(Wqkv, np.float32)
    Wproj = np.asarray(Wproj, np.float32)
    bproj = np.asarray(bproj, np.float32)
    out, _ = _run(x, Wqkv, Wproj, bproj)
    return out
